# revision 27
# baseline (speedup 1.0000x reference)
"""ChebNetII (gnn_message_passing) on 8 Trainium2 NeuronCores.

kernel(**inputs) takes the FULL inputs and returns the FULL [100000, 64]
fp32 output.

Adaptive structure: out = sum_i coe_i * T_i(L_tilde) h with
coe = 2/(K+1) * M @ temp computed on host in fp64. Terms with
|coe_i| below 1e-6 * max|coe| contribute nothing at the 2e-2 tolerance,
so the kernel only runs propagation steps up to the largest significant
order K_eff. With the default temp=ones initialization the Chebyshev-node
discrete orthogonality makes every coe_i (i>=1) vanish identically, so
K_eff == 0 and the kernel reduces to the MLP: a small fp16 matmul
pipeline (x@W1 relu @W2*s + b2*s), node-sharded 8 ways.

For K_eff >= 1 the full gather/segment-sum propagation path from the
baseline runs (shard dst nodes, AllGather fp16 messages, indirect-DMA
gather by edge slot, PE block-ones segment sums), truncated at K_eff.
"""
import sys
sys.path.insert(0, '/opt/trn_rl_repo')
import numpy as np

# ---------------------------------------------------------------------------
# problem constants (hardcoded per the harness contract)
# ---------------------------------------------------------------------------
N = 100000
E = 1600000
P = 8
NP = N // P            # 12500
SHARD = 12544          # 98 * 128
F_IN = 256
HID = 64
K = 10
L = 4                  # edge slots per vrow
PSUM_VIDS = 1024       # vids per psum tile (4 matmuls x 8 groups x 32 vids)
PAD_IDX = SHARD - 1    # core0 pad row: deg==0 -> dis==0 -> u row is zeros
NCH = SHARD // 128     # 98


# ---------------------------------------------------------------------------
# toolchain workarounds (this walrus build rejects multi-wait instructions)
# ---------------------------------------------------------------------------
_SLIM_EXIT = True


def _install_patches():
    import concourse.tile as tile
    import concourse.mybir as mybir
    from concourse.vector_clock import ScopedClock

    def _patched_drain_and_barrier(self, tick_clock, wait_clock):
        nc = self.nc
        drain_inst = nc.sync.drain()
        wait_clock.add_sem_waits(
            drain_inst.ins, ScopedClock({None: tick_clock.global_clock})
        )
        si = drain_inst.ins.sync_info
        if si is not None and si.on_wait and len(si.on_wait) > 1:
            waits = list(si.on_wait)
            si.on_wait = waits[:1]
            for w in waits[1:]:
                nop = nc.sync.nop(nofuse=True, hint="drain_wait_spill")
                nop.ins.sync_info = mybir.SyncInfo(on_wait=[w], on_update=[])
        nc.all_engine_barrier()
        assert self.sems is not None
        popped = nc._tile_sem_poison_stack.pop()
        assert popped is self._sem_poison
        if not _SLIM_EXIT:
            # semaphore state is dead once the program ends; skip the cleanup
            # instructions and the second barrier to shorten the tail
            nc.clear_and_free_semaphores(list(self.sems.allocated().values()))
            nc.all_engine_barrier()

    tile.TileContext._drain_and_barrier = _patched_drain_and_barrier


def _legalize_waits(nc, max_waits=1):
    import concourse.mybir as mybir
    for fn in nc.m.functions:
        for bb in fn.blocks:
            new_insts = []
            for inst in bb.instructions:
                si = inst.sync_info
                if si is not None and si.on_wait and len(si.on_wait) > max_waits:
                    waits = list(si.on_wait)
                    si.on_wait = waits[:max_waits]
                    extra = waits[max_waits:]
                    for i in range(0, len(extra), max_waits):
                        nop = mybir.InstNoOp(
                            name=nc.get_next_instruction_name(),
                            engine=inst.engine,
                            ins=[], outs=[],
                            bass_nofuse=True,
                            text_hint="wait_spill",
                            sync_info=mybir.SyncInfo(
                                on_wait=extra[i:i + max_waits], on_update=[]),
                        )
                        nc.register_instruction(nop, overwrite=True)
                        new_insts.append(nop)
                new_insts.append(inst)
            bb.instructions[:] = new_insts


def _dedupe_ldweights(nc):
    """Drop InstLdweights that reload the PE array with the very weights it
    already holds (same AP/tile/perf-mode, no semaphore payload). Runs on the
    final scheduled instruction order."""
    for fn in nc.m.functions:
        for bb in fn.blocks:
            cur_sig = None
            keep = []
            for inst in bb.instructions:
                tn = type(inst).__name__
                if tn == 'InstLdweights':
                    si = inst.sync_info
                    has_sync = bool(si and (si.on_wait or si.on_update))
                    sig = (str(inst.ins[0]),
                           str(getattr(inst, 'tile_position', None)),
                           str(getattr(inst, 'perf_mode', None)),
                           str(getattr(inst, 'is_transpose', None)))
                    if sig == cur_sig and not has_sync:
                        continue
                    cur_sig = sig
                keep.append(inst)
            bb.instructions[:] = keep


# ---------------------------------------------------------------------------
# Chebyshev coefficient helpers (host, fp64)
# ---------------------------------------------------------------------------
def _cheb_M64():
    j = np.arange(K + 1)
    xs = np.cos((K - j + 0.5) * np.pi / (K + 1))
    M = np.zeros((K + 1, K + 1), dtype=np.float64)
    M[0] = 1.0
    M[1] = xs
    for i in range(2, K + 1):
        M[i] = 2.0 * xs * M[i - 1] - M[i - 2]
    return M


def _cheb_MT():
    return np.ascontiguousarray(
        (2.0 / (K + 1)) * _cheb_M64().astype(np.float32).T)


# ---------------------------------------------------------------------------
# fast path: K_eff == 0  ->  out = s * (relu(x@W1+b1)@W2+b2), s = coe0/2
# ---------------------------------------------------------------------------
# input x is packed on host as xH [128, 25088] f16:
#   xH[p, 1024*j + nw_j*k + c] = x[512*j + c, 128*k + p]
# so each 512-node block j is 1024 contiguous cols (two 512-col contraction
# chunks). DMA chunks double as mm1 groups: small chunks first so the
# pipeline fills fast, 8KB-per-partition descriptors mid-stream.
_BLK = [512] * 24 + [256]
_BASE = [1024 * j for j in range(24)] + [24576]
_CHUNKS = ([[0], [1], [2], [3], [4]] +
           [[j, j + 1] for j in range(5, 24, 2)])
# (after block, col lo, col hi): fire each output chunk as soon as its last
# block's bias lands; finer splits near the end to shorten the tail
_OUT_SPLITS = [(9, 0, 5120), (13, 5120, 7168), (17, 7168, 9216),
               (21, 9216, 11264), (23, 11264, 12288), (24, 12288, 12544)]

_CHUNK_OF = {}
for _ci, _blks in enumerate(_CHUNKS):
    for _k, _j in enumerate(_blks):
        _CHUNK_OF[_j] = (_ci, _BASE[_j] - _BASE[_blks[0]])


def _build_mlp_bass():
    import concourse.bass as bass
    import concourse.mybir as mybir
    import concourse.tile as tile

    F32 = mybir.dt.float32
    F16 = mybir.dt.float16
    AF = mybir.ActivationFunctionType

    nc = bass.Bass()
    xH_d = nc.dram_tensor("xH", [128, 25088], F16, kind="ExternalInput")
    Wp_d = nc.dram_tensor("Wp", [128, 192], F16, kind="ExternalInput")
    bp_d = nc.dram_tensor("bp", [64, 2], F32, kind="ExternalInput")
    out_d = nc.dram_tensor("out", [64, SHARD], F16, kind="ExternalOutput")

    with tile.TileContext(nc) as tc:
        with tc.tile_pool(name="const", bufs=1) as const, \
             tc.tile_pool(name="xs", bufs=6) as xs, \
             tc.tile_pool(name="hs", bufs=5) as hs, \
             tc.tile_pool(name="ps", bufs=8, space="PSUM") as ps_pool:

            Wt = const.tile([128, 192], F16, tag="W")
            bt = const.tile([64, 2], F32, tag="b")
            ho = []
            for i, (_, lo, hi) in enumerate(_OUT_SPLITS):
                ho_i = const.tile([64, hi - lo], F16, tag=f"ho{i}",
                                  name=f"ho{i}")
                ho.append(ho_i)

            xts = []

            def issue_chunk(ci):
                blks = _CHUNKS[ci]
                c0 = _BASE[blks[0]]
                cw = _BASE[blks[-1]] + 2 * _BLK[blks[-1]] - c0
                xt = xs.tile([128, 2048], F16, tag="xt", name=f"xt{ci}")
                nc.sync.dma_start(xt[:, :cw], xH_d[:, c0:c0 + cw])
                xts.append(xt)

            # x chunk 0 first (critical path), then the consts, then prefetch
            issue_chunk(0)
            nc.sync.dma_start(Wt[:], Wp_d[:])
            nc.sync.dma_start(bt[:], bp_d[:])
            issue_chunk(1)
            issue_chunk(2)

            def mm(out, lhsT, rhs, start, stop, reload):
                m = nc.tensor.matmul(out, lhsT=lhsT, rhs=rhs,
                                     start=start, stop=stop)
                if not reload:
                    m.ins.ldweights = False
                return m

            def bias_out(j, ps2s, use_scalar=False):
                # bias add for block j + out DMA when its staging chunk
                # completes. Output DMAs ride the GpSimd ring so they never
                # block input chunks on the sync ring (head-of-line).
                nw = _BLK[j]
                col = 512 * j
                for i, (after, lo, hi) in enumerate(_OUT_SPLITS):
                    if lo <= col < hi:
                        dst = ho[i][:, col - lo:col - lo + nw]
                        if use_scalar:
                            nc.scalar.activation(dst, ps2s[j][:, :nw],
                                                 AF.Identity, bias=bt[:, 1:2])
                        else:
                            nc.vector.tensor_scalar_add(dst, ps2s[j][:, :nw],
                                                        bt[:, 1:2])
                        if j == after:
                            nc.gpsimd.dma_start(out_d[:, lo:hi], ho[i][:])
                        break

            def emit_mm2(grp, h1s, ps2s, tail=False):
                # mm2 batch for a finished group, then bias adds + out DMAs.
                # In the tail (no relus left to delay) alternate DVE/scalar
                # to halve the serial bias chain.
                for k, j in enumerate(grp):
                    nw = _BLK[j]
                    ps2 = ps_pool.tile([64, 512], F32, tag="ps")
                    mm(ps2[:, :nw], Wt[0:64, 128:192], h1s[j][:, :nw],
                       True, True, k == 0)
                    ps2s[j] = ps2
                for k, j in enumerate(grp):
                    bias_out(j, ps2s, use_scalar=tail and k % 2 == 1)

            # mm1 groups of 4 blocks (2 chunks); mm2 of the previous group
            # is issued after the full mm1 pair phases (v3 ordering)
            groups = [list(range(g, min(g + 4, 25))) for g in range(0, 25, 4)]
            h1s = [None] * 25
            ps1s = [None] * 25
            ps2s = [None] * 25
            next_chunk = 3
            for gi, grp in enumerate(groups):
                while next_chunk <= min(_CHUNK_OF[grp[-1]][0] + 2,
                                        len(_CHUNKS) - 1):
                    issue_chunk(next_chunk)
                    next_chunk += 1
                for k, j in enumerate(grp):
                    nw = _BLK[j]
                    ci, off = _CHUNK_OF[j]
                    ps1 = ps_pool.tile([64, 512], F32, tag="ps")
                    ps1s[j] = ps1
                    mm(ps1[:, :nw], Wt[:, 0:64], xts[ci][:, off:off + nw],
                       True, False, k == 0)
                for k, j in enumerate(grp):
                    nw = _BLK[j]
                    ci, off = _CHUNK_OF[j]
                    mm(ps1s[j][:, :nw], Wt[:, 64:128],
                       xts[ci][:, off + nw:off + 2 * nw], False, True, k == 0)
                    h1 = hs.tile([64, 512], F16, tag="h1")
                    nc.scalar.activation(h1[:, :nw], ps1s[j][:, :nw], AF.Relu,
                                         bias=bt[:, 0:1])
                    h1s[j] = h1
                if gi >= 1:
                    emit_mm2(groups[gi - 1], h1s, ps2s,
                             tail=(gi == len(groups) - 1))
            emit_mm2(groups[-1], h1s, ps2s, tail=True)

    _dedupe_ldweights(nc)
    _legalize_waits(nc)
    return nc


def _mlp_kernel(x, W1, b1, W2, b2, scale):
    from concourse.bass_utils import run_bass_kernel_spmd
    nc = _build_mlp_bass()

    W1r = np.ascontiguousarray(
        W1.reshape(2, 128, 64).transpose(1, 0, 2).reshape(128, 128)
    ).astype(np.float16)
    W2s = (W2 * scale).astype(np.float16)
    Wp = np.zeros((128, 192), np.float16)
    Wp[:, 0:128] = W1r
    Wp[0:64, 128:192] = W2s
    bp = np.stack([b1, b2 * scale], axis=1).astype(np.float32)

    maps = []
    for c in range(P):
        xp = np.zeros((SHARD, 256), np.float16)
        xp[:NP] = x[c * NP:(c + 1) * NP]
        xH = np.empty((128, 25088), np.float16)
        xH[:, :24576] = (xp[:12288].reshape(24, 512, 2, 128)
                         .transpose(3, 0, 2, 1).reshape(128, 24576))
        xH[:, 24576:] = (xp[12288:].reshape(256, 2, 128)
                         .transpose(2, 1, 0).reshape(128, 512))
        maps.append({"xH": np.ascontiguousarray(xH), "Wp": Wp, "bp": bp})

    res = run_bass_kernel_spmd(nc, maps, core_ids=list(range(P)))

    full = np.empty((N, HID), np.float32)
    for c in range(P):
        full[c * NP:(c + 1) * NP] = res.results[c]["out"].T[:NP].astype(np.float32)
    return full


# ---------------------------------------------------------------------------
# general path: host-side graph preprocessing
# ---------------------------------------------------------------------------
def _vid_to_slotbase(v):
    t = v // 1024
    q = (v % 1024) // 128
    j = (v % 128) // 32
    m = v % 32
    return (32 * t + 8 * j + q) * 128 + 4 * m


def _build_structures(edge_index):
    rows = np.asarray(edge_index[0], dtype=np.int64)
    cols = np.asarray(edge_index[1], dtype=np.int64)
    outdeg = np.bincount(rows, minlength=N)

    cores = []
    for c in range(P):
        lo = c * NP
        sel = (cols >= lo) & (cols < lo + NP)
        e_src = rows[sel]
        e_dst = cols[sel] - lo
        order = np.argsort(e_dst, kind="stable")
        e_src = e_src[order]
        indeg = np.bincount(e_dst, minlength=NP)
        starts = np.zeros(NP + 1, dtype=np.int64)
        np.cumsum(indeg, out=starts[1:])
        vcnt = np.maximum(1, -(-indeg // L))
        perm = np.argsort(vcnt, kind="stable")
        cores.append(dict(e_src=e_src, starts=starts, indeg=indeg,
                          vcnt=vcnt, perm=perm))

    max_vc = max(int(c["vcnt"].max()) for c in cores)
    sizes = [SHARD]
    for p in range(1, max_vc):
        a = max(int((c["vcnt"] > p).sum()) for c in cores)
        sizes.append(min(SHARD, -(-(a + SHARD - NP) // 128) * 128))
    bases = np.concatenate([[0], np.cumsum(sizes)[:-1]]).astype(np.int64)
    acc_starts = np.array([0] + [SHARD - s for s in sizes[1:]], dtype=np.int64)
    NVID = int(sum(sizes))
    NVID_pad = -(-NVID // PSUM_VIDS) * PSUM_VIDS
    NSLOT = NVID_pad * L

    perm_pos = np.empty((P, NP), dtype=np.int64)
    for c in range(P):
        perm_pos[c][cores[c]["perm"]] = np.arange(NP)
    g_row = (np.repeat(np.arange(P), NP) * SHARD + perm_pos.ravel())

    all_idx, all_mask = [], []
    for c in range(P):
        cc = cores[c]
        idx = np.full(NSLOT, PAD_IDX, dtype=np.int32)
        for p in range(len(sizes)):
            sz, b, astart = sizes[p], int(bases[p]), int(acc_starts[p])
            r = np.arange(astart, astart + sz)
            v = b + (r - astart)
            real = r < NP
            d = cc["perm"][np.minimum(r, NP - 1)]
            has = real & (cc["vcnt"][d] > p)
            d_sel, v_sel = d[has], v[has]
            sbase = _vid_to_slotbase(v_sel)
            estart = cc["starts"][d_sel] + p * L
            cnt = np.minimum(cc["starts"][d_sel] + cc["indeg"][d_sel],
                             estart + L) - estart
            for i in range(L):
                sub = cnt > i
                src = cc["e_src"][estart[sub] + i]
                idx[sbase[sub] + i] = g_row[src]
        all_idx.append(idx)
        od = np.zeros(SHARD, dtype=np.int64)
        od[:NP] = outdeg[c * NP + cc["perm"]]
        all_mask.append((np.arange(64)[None, :] < od[:, None]).astype(np.float16))

    plan = dict(sizes=sizes, bases=bases, acc_starts=acc_starts,
                NVID=NVID, NVID_pad=NVID_pad, NSLOT=NSLOT)
    return cores, all_idx, all_mask, plan


def _plane_of_vid(plan, v0):
    bases, sizes = plan["bases"], plan["sizes"]
    p = int(np.searchsorted(bases, v0, side="right")) - 1
    if v0 >= bases[p] + sizes[p]:
        return None
    return p


def _dve_schedule(plan):
    ops = []
    n_tiles = plan["NVID_pad"] // PSUM_VIDS
    for t in range(n_tiles):
        run = None
        for q in range(8):
            v0 = 1024 * t + 128 * q
            p = _plane_of_vid(plan, v0) if v0 < plan["NVID"] else None
            if p is None:
                if run is not None:
                    ops.append(run)
                    run = None
                continue
            acc_row = int(plan["acc_starts"][p]) + (v0 - int(plan["bases"][p]))
            is_copy, chunk = (p == 0), acc_row // 128
            if (run is not None and run[3] == is_copy
                    and run[4] + (q - run[1]) == chunk):
                run = (t, run[1], q + 1, is_copy, run[4])
            else:
                if run is not None:
                    ops.append(run)
                run = (t, q, q + 1, is_copy, chunk)
        if run is not None:
            ops.append(run)
    return ops


# ---------------------------------------------------------------------------
# general-path Bass program (propagation up to k_eff steps)
# ---------------------------------------------------------------------------
def _build_bass(plan, sched, k_eff):
    import concourse.bass as bass
    import concourse.mybir as mybir
    import concourse.tile as tile
    from concourse.bass import IndirectOffsetOnAxis

    F32 = mybir.dt.float32
    F16 = mybir.dt.float16
    I32 = mybir.dt.int32
    AF = mybir.ActivationFunctionType
    OP = mybir.AluOpType

    NSLOT = plan["NSLOT"]
    groups_used = plan["NVID_pad"] // 32
    n_chunks = -(-groups_used // 128)
    n_ptiles = -(-groups_used // 32)
    sched_by_tile = {}
    for op in sched:
        sched_by_tile.setdefault(op[0], []).append(op)

    nc = bass.Bass()
    xT_d = nc.dram_tensor("xT", [256, SHARD], F32, kind="ExternalInput")
    W1_d = nc.dram_tensor("W1", [256, 64], F32, kind="ExternalInput")
    b1_d = nc.dram_tensor("b1", [64, 1], F32, kind="ExternalInput")
    W2_d = nc.dram_tensor("W2", [64, 64], F32, kind="ExternalInput")
    b2_d = nc.dram_tensor("b2", [64, 1], F32, kind="ExternalInput")
    chebMT_d = nc.dram_tensor("chebMT", [11, 11], F32, kind="ExternalInput")
    temp_d = nc.dram_tensor("temp", [11, 1], F32, kind="ExternalInput")
    ident_d = nc.dram_tensor("ident", [64, 64], F32, kind="ExternalInput")
    ones1_d = nc.dram_tensor("ones1", [128, 32], F16, kind="ExternalInput")
    ones2_d = nc.dram_tensor("ones2", [128, 32], F16, kind="ExternalInput")
    gidx_d = nc.dram_tensor("gidx", [128, NSLOT // 128], I32, kind="ExternalInput")
    mask_d = nc.dram_tensor("maskd", [SHARD, 64], F16, kind="ExternalInput")
    out_d = nc.dram_tensor("out", [SHARD, 64], F32, kind="ExternalOutput")

    with tile.TileContext(nc) as tc:
        with tc.tile_pool(name="big", bufs=1) as big, \
             tc.tile_pool(name="msgs", bufs=2) as msgs_pool, \
             tc.tile_pool(name="ps", bufs=4, space="PSUM") as ps_pool, \
             tc.tile_pool(name="sm", bufs=3) as sm, \
             tc.tile_pool(name="dram", bufs=1, space="DRAM") as dram:

            TxA = big.tile([128, NCH, 64], F32, tag="TxA")
            TxB = big.tile([128, NCH, 64], F32, tag="TxB")
            acc = big.tile([128, NCH, 64], F32, tag="acc")
            oacc = big.tile([128, NCH, 64], F32, tag="oacc")
            disw = big.tile([128, NCH, 64], F32, tag="disw")
            u16 = big.tile([128, NCH, 64], F16, tag="u16")
            idxt = big.tile([128, NSLOT // 128], I32, tag="idx")
            ones1 = big.tile([128, 32], F16, tag="ones1")
            ones2 = big.tile([128, 32], F16, tag="ones2")
            onesf = big.tile([128, 64], F32, tag="onesf")
            ones1x = big.tile([1, 128], F32, tag="ones1x")
            identt = big.tile([64, 64], F32, tag="ident")
            W1t = big.tile([128, 2, 64], F32, tag="W1")
            W2t = big.tile([64, 64], F32, tag="W2")
            b1t = big.tile([64, 1], F32, tag="b1")
            b2t = big.tile([64, 1], F32, tag="b2")
            coe_t = big.tile([128, 11], F32, tag="coe")
            dis_t = big.tile([128, NCH], F32, tag="dis")
            m1_t = big.tile([128, NCH], F32, tag="m1")

            nc.sync.dma_start(idxt[:], gidx_d[:])
            nc.sync.dma_start(W1t[:], W1_d[:].rearrange("(k p) h -> p k h", p=128))
            nc.sync.dma_start(W2t[:], W2_d[:])
            nc.sync.dma_start(b1t[:], b1_d[:])
            nc.sync.dma_start(b2t[:], b2_d[:])
            nc.sync.dma_start(identt[:], ident_d[:])
            nc.sync.dma_start(ones1[:], ones1_d[:])
            nc.sync.dma_start(ones2[:], ones2_d[:])
            nc.vector.memset(onesf[:], 1.0)
            nc.vector.memset(ones1x[:], 1.0)

            # coe = (2/(K+1)) * M @ temp, broadcast to all 128 partitions
            chebt = sm.tile([11, 11], F32, tag="chebt")
            tempt = sm.tile([11, 1], F32, tag="tempt")
            nc.sync.dma_start(chebt[:], chebMT_d[:])
            nc.sync.dma_start(tempt[:], temp_d[:])
            ps_coe = ps_pool.tile([1, 11], F32, tag="ps")
            nc.tensor.matmul(ps_coe[:], lhsT=tempt[:], rhs=chebt[:], start=True, stop=True)
            coe_row = sm.tile([1, 11], F32, tag="coerow")
            nc.vector.tensor_copy(coe_row[:], ps_coe[:])
            ps_coeb = ps_pool.tile([128, 11], F32, tag="ps")
            nc.tensor.matmul(ps_coeb[:], lhsT=ones1x[:], rhs=coe_row[:], start=True, stop=True)
            nc.vector.tensor_copy(coe_t[:], ps_coeb[:])

            # deg/dis from the out-degree unary mask
            maskt = msgs_pool.tile([128, NCH, 64], F16, tag="msgs")
            nc.sync.dma_start(maskt[:], mask_d[:].rearrange("(c p) f -> p c f", p=128))
            deg = sm.tile([128, NCH], F32, tag="deg")
            nc.vector.tensor_reduce(deg[:], maskt[:], axis=mybir.AxisListType.X, op=OP.add)
            nc.vector.tensor_scalar_min(m1_t[:], deg[:], 1.0)
            nc.vector.tensor_scalar_max(deg[:], deg[:], 0.5)
            rec = sm.tile([128, NCH], F32, tag="rec")
            nc.vector.reciprocal(rec[:], deg[:])
            nc.scalar.activation(dis_t[:], rec[:], AF.Sqrt)
            nc.vector.tensor_tensor(out=dis_t[:], in0=dis_t[:], in1=m1_t[:], op=OP.mult)
            for c in range(NCH):
                nc.scalar.activation(disw[:, c, :], onesf[:], AF.Copy,
                                     scale=dis_t[:, c:c + 1])

            # MLP: h = relu(x@W1+b1)@W2+b2, node-major into TxA
            nco = 0
            ci = 0
            for j in range(25):
                nw = 512 if j < 24 else 256
                ps1 = ps_pool.tile([64, 512], F32, tag="ps")
                for k in range(2):
                    xt = sm.tile([128, 512], F32, tag="xt")
                    nc.sync.dma_start(xt[:, :nw], xT_d[128 * k:128 * (k + 1), nco:nco + nw])
                    nc.tensor.matmul(ps1[:, :nw], lhsT=W1t[:, k, :], rhs=xt[:, :nw],
                                     start=(k == 0), stop=(k == 1))
                h1 = sm.tile([64, 512], F32, tag="h1")
                nc.scalar.activation(h1[:, :nw], ps1[:, :nw], AF.Relu, bias=b1t[:, 0:1])
                ps2 = ps_pool.tile([64, 512], F32, tag="ps")
                nc.tensor.matmul(ps2[:, :nw], lhsT=W2t[:], rhs=h1[:, :nw], start=True, stop=True)
                h2 = sm.tile([64, 512], F32, tag="h2")
                nc.vector.tensor_scalar_add(h2[:, :nw], ps2[:, :nw], b2t[:, 0:1])
                for cc in range(nw // 128):
                    pst = ps_pool.tile([128, 64], F32, tag="ps")
                    nc.tensor.transpose(pst[:], h2[:, 128 * cc:128 * (cc + 1)], identt[:])
                    nc.vector.tensor_copy(TxA[:, ci, :], pst[:])
                    ci += 1
                nco += nw

            # Chebyshev propagation steps
            u_bounce = dram.tile([SHARD, 64], F16, tag="ub")
            cur, prev = TxA, TxB
            for s in range(1, k_eff + 1):
                nc.vector.tensor_tensor(out=u16[:], in0=cur[:], in1=disw[:], op=OP.mult)
                nc.sync.dma_start(u_bounce[:].rearrange("(c p) f -> p c f", p=128), u16[:])
                ufull = dram.tile([P * SHARD, 64], F16, addr_space="Shared", tag=f"uf{s}")
                nc.gpsimd.collective_compute(
                    "AllGather", OP.bypass,
                    replica_groups=[list(range(P))],
                    ins=[u_bounce.opt()], outs=[ufull.opt()],
                )
                ones_t = ones1 if s == 1 else ones2
                for kk in range(n_chunks):
                    g0 = 128 * kk
                    gn = min(128, groups_used - g0)
                    mt = msgs_pool.tile([128, 128 * 64], F16, tag="msgs")
                    nc.gpsimd.indirect_dma_start(
                        out=mt[:, :gn * 64], out_offset=None,
                        in_=ufull[:],
                        in_offset=IndirectOffsetOnAxis(ap=idxt[:, g0:g0 + gn], axis=0),
                    )
                    for tt in range(4):
                        T = 4 * kk + tt
                        if T >= n_ptiles:
                            break
                        ps = ps_pool.tile([128, 512], F32, tag="ps")
                        for jj in range(4):
                            gbase = 32 * tt + 8 * jj
                            nq = min(8, groups_used - (32 * T + 8 * jj))
                            if nq <= 0:
                                break
                            nc.tensor.matmul(ps[32 * jj:32 * (jj + 1), :64 * nq],
                                             lhsT=ones_t[:],
                                             rhs=mt[:, gbase * 64:(gbase + nq) * 64],
                                             start=True, stop=True,
                                             tile_position=(0, 32 * jj))
                        for (_, qlo, qhi, is_copy, ch0) in sched_by_tile.get(T, []):
                            src = ps[:, 64 * qlo:64 * qhi]
                            dst = acc[:, ch0:ch0 + (qhi - qlo), :]
                            if is_copy:
                                nc.vector.tensor_copy(dst, src)
                            else:
                                nc.vector.tensor_tensor(out=dst, in0=dst, in1=src, op=OP.add)
                nc.vector.tensor_tensor(out=acc[:], in0=acc[:], in1=disw[:], op=OP.mult)
                if s == 1:
                    nc.vector.tensor_copy(prev[:], acc[:])
                    nc.vector.tensor_scalar(out=oacc[:], in0=cur[:],
                                            scalar1=coe_t[:, 0:1], scalar2=0.5,
                                            op0=OP.mult, op1=OP.mult)
                    nc.vector.tensor_scalar(out=acc[:], in0=prev[:],
                                            scalar1=coe_t[:, 1:2], scalar2=None,
                                            op0=OP.mult)
                    nc.vector.tensor_tensor(out=oacc[:], in0=oacc[:], in1=acc[:], op=OP.add)
                else:
                    nc.vector.tensor_tensor(out=prev[:], in0=acc[:], in1=prev[:], op=OP.subtract)
                    nc.vector.tensor_scalar(out=acc[:], in0=prev[:],
                                            scalar1=coe_t[:, s:s + 1], scalar2=None,
                                            op0=OP.mult)
                    nc.vector.tensor_tensor(out=oacc[:], in0=oacc[:], in1=acc[:], op=OP.add)
                cur, prev = prev, cur

            if k_eff == 0:
                nc.vector.tensor_scalar(out=oacc[:], in0=TxA[:],
                                        scalar1=coe_t[:, 0:1], scalar2=0.5,
                                        op0=OP.mult, op1=OP.mult)
            nc.sync.dma_start(out_d[:].rearrange("(c p) f -> p c f", p=128), oacc[:])

    _legalize_waits(nc)
    return nc


def _block_ones(v):
    o = np.zeros((128, 32), np.float16)
    for m in range(32):
        o[4 * m:4 * m + 4, m] = v
    return o


def _general_kernel(x, edge_index, W1, b1, W2, b2, temp, k_eff):
    from concourse.bass_utils import run_bass_kernel_spmd

    cores, all_idx, all_mask, plan = _build_structures(edge_index)
    sched = _dve_schedule(plan)
    nc = _build_bass(plan, sched, k_eff)

    chebMT = _cheb_MT()
    ident = np.eye(64, dtype=np.float32)
    o1, o2 = _block_ones(-1.0), _block_ones(-2.0)
    maps = []
    for c in range(P):
        cc = cores[c]
        xp = x[c * NP + cc["perm"]]
        xp = np.concatenate([xp, np.zeros((SHARD - NP, 256), np.float32)])
        maps.append({
            "xT": np.ascontiguousarray(xp.T),
            "W1": W1, "b1": b1.reshape(64, 1),
            "W2": W2, "b2": b2.reshape(64, 1),
            "chebMT": chebMT,
            "temp": temp.reshape(11, 1),
            "ident": ident,
            "ones1": o1, "ones2": o2,
            "gidx": np.ascontiguousarray(all_idx[c].reshape(-1, 128).T),
            "maskd": all_mask[c],
        })

    res = run_bass_kernel_spmd(nc, maps, core_ids=list(range(P)))

    full = np.zeros((N, HID), np.float32)
    for c in range(P):
        full[c * NP + cores[c]["perm"]] = res.results[c]["out"][:NP]
    return full


# ---------------------------------------------------------------------------
# public entry point
# ---------------------------------------------------------------------------
def kernel(x, edge_index, W1, b1, W2, b2, temp):
    _install_patches()

    x = np.asarray(x, np.float32)
    W1 = np.asarray(W1, np.float32)
    b1 = np.asarray(b1, np.float32)
    W2 = np.asarray(W2, np.float32)
    b2 = np.asarray(b2, np.float32)
    temp = np.asarray(temp, np.float32)

    # significant Chebyshev orders, computed on host in fp64
    coe = (2.0 / (K + 1)) * (_cheb_M64() @ temp.astype(np.float64))
    thr = 1e-6 * max(np.abs(coe).max(), 1e-30)
    sig = np.nonzero(np.abs(coe) > thr)[0]
    k_eff = int(sig.max()) if (sig.size and sig.max() >= 1) else 0

    if k_eff == 0:
        return _mlp_kernel(x, W1, b1, W2, b2, float(coe[0] / 2.0))
    return _general_kernel(x, edge_index, W1, b1, W2, b2, temp, k_eff)


# revision 29
# speedup vs baseline: 1.0438x; 1.0438x over previous
"""ChebNetII (gnn_message_passing) on 8 Trainium2 NeuronCores.

kernel(**inputs) takes the FULL inputs and returns the FULL [100000, 64]
fp32 output.

Adaptive structure: out = sum_i coe_i * T_i(L_tilde) h with
coe = 2/(K+1) * M @ temp computed on host in fp64. Terms with
|coe_i| below 1e-6 * max|coe| contribute nothing at the 2e-2 tolerance,
so the kernel only runs propagation steps up to the largest significant
order K_eff. With the default temp=ones initialization the Chebyshev-node
discrete orthogonality makes every coe_i (i>=1) vanish identically, so
K_eff == 0 and the kernel reduces to the MLP: a small fp16 matmul
pipeline (x@W1 relu @W2*s + b2*s), node-sharded 8 ways.

For K_eff >= 1 the full gather/segment-sum propagation path from the
baseline runs (shard dst nodes, AllGather fp16 messages, indirect-DMA
gather by edge slot, PE block-ones segment sums), truncated at K_eff.
"""
import sys
sys.path.insert(0, '/opt/trn_rl_repo')
import numpy as np

# ---------------------------------------------------------------------------
# problem constants (hardcoded per the harness contract)
# ---------------------------------------------------------------------------
N = 100000
E = 1600000
P = 8
NP = N // P            # 12500
SHARD = 12544          # 98 * 128
F_IN = 256
HID = 64
K = 10
L = 4                  # edge slots per vrow
PSUM_VIDS = 1024       # vids per psum tile (4 matmuls x 8 groups x 32 vids)
PAD_IDX = SHARD - 1    # core0 pad row: deg==0 -> dis==0 -> u row is zeros
NCH = SHARD // 128     # 98


# ---------------------------------------------------------------------------
# toolchain workarounds (this walrus build rejects multi-wait instructions)
# ---------------------------------------------------------------------------
_SLIM_EXIT = True


def _install_patches():
    import concourse.tile as tile
    import concourse.mybir as mybir
    from concourse.vector_clock import ScopedClock

    def _patched_drain_and_barrier(self, tick_clock, wait_clock):
        nc = self.nc
        drain_inst = nc.sync.drain()
        wait_clock.add_sem_waits(
            drain_inst.ins, ScopedClock({None: tick_clock.global_clock})
        )
        si = drain_inst.ins.sync_info
        if si is not None and si.on_wait and len(si.on_wait) > 1:
            waits = list(si.on_wait)
            si.on_wait = waits[:1]
            for w in waits[1:]:
                nop = nc.sync.nop(nofuse=True, hint="drain_wait_spill")
                nop.ins.sync_info = mybir.SyncInfo(on_wait=[w], on_update=[])
        nc.all_engine_barrier()
        assert self.sems is not None
        popped = nc._tile_sem_poison_stack.pop()
        assert popped is self._sem_poison
        if not _SLIM_EXIT:
            # semaphore state is dead once the program ends; skip the cleanup
            # instructions and the second barrier to shorten the tail
            nc.clear_and_free_semaphores(list(self.sems.allocated().values()))
            nc.all_engine_barrier()

    tile.TileContext._drain_and_barrier = _patched_drain_and_barrier


def _legalize_waits(nc, max_waits=1):
    import concourse.mybir as mybir
    for fn in nc.m.functions:
        for bb in fn.blocks:
            new_insts = []
            for inst in bb.instructions:
                si = inst.sync_info
                if si is not None and si.on_wait and len(si.on_wait) > max_waits:
                    waits = list(si.on_wait)
                    si.on_wait = waits[:max_waits]
                    extra = waits[max_waits:]
                    for i in range(0, len(extra), max_waits):
                        nop = mybir.InstNoOp(
                            name=nc.get_next_instruction_name(),
                            engine=inst.engine,
                            ins=[], outs=[],
                            bass_nofuse=True,
                            text_hint="wait_spill",
                            sync_info=mybir.SyncInfo(
                                on_wait=extra[i:i + max_waits], on_update=[]),
                        )
                        nc.register_instruction(nop, overwrite=True)
                        new_insts.append(nop)
                new_insts.append(inst)
            bb.instructions[:] = new_insts


def _dedupe_ldweights(nc):
    """Drop InstLdweights that reload the PE array with the very weights it
    already holds (same AP/tile/perf-mode, no semaphore payload). Runs on the
    final scheduled instruction order."""
    for fn in nc.m.functions:
        for bb in fn.blocks:
            cur_sig = None
            keep = []
            for inst in bb.instructions:
                tn = type(inst).__name__
                if tn == 'InstLdweights':
                    si = inst.sync_info
                    has_sync = bool(si and (si.on_wait or si.on_update))
                    sig = (str(inst.ins[0]),
                           str(getattr(inst, 'tile_position', None)),
                           str(getattr(inst, 'perf_mode', None)),
                           str(getattr(inst, 'is_transpose', None)))
                    if sig == cur_sig and not has_sync:
                        continue
                    cur_sig = sig
                keep.append(inst)
            bb.instructions[:] = keep


# ---------------------------------------------------------------------------
# Chebyshev coefficient helpers (host, fp64)
# ---------------------------------------------------------------------------
def _cheb_M64():
    j = np.arange(K + 1)
    xs = np.cos((K - j + 0.5) * np.pi / (K + 1))
    M = np.zeros((K + 1, K + 1), dtype=np.float64)
    M[0] = 1.0
    M[1] = xs
    for i in range(2, K + 1):
        M[i] = 2.0 * xs * M[i - 1] - M[i - 2]
    return M


def _cheb_MT():
    return np.ascontiguousarray(
        (2.0 / (K + 1)) * _cheb_M64().astype(np.float32).T)


# ---------------------------------------------------------------------------
# fast path: K_eff == 0  ->  out = s * (relu(x@W1+b1)@W2+b2), s = coe0/2
# ---------------------------------------------------------------------------
# input x is packed on host as xH [128, 25088] f16:
#   xH[p, 1024*j + nw_j*k + c] = x[512*j + c, 128*k + p]
# so each 512-node block j is 1024 contiguous cols (two 512-col contraction
# chunks). DMA chunks double as mm1 groups: small chunks first so the
# pipeline fills fast, 8KB-per-partition descriptors mid-stream.
_BLK = [512] * 24 + [256]
_BASE = [1024 * j for j in range(24)] + [24576]
_CHUNKS = [[0]] + [[j, j + 1] for j in range(1, 24, 2)]
# (after block, col lo, col hi): fire each output chunk as soon as its last
# block's bias lands; finer splits near the end to shorten the tail
_OUT_SPLITS = [(9, 0, 5120), (13, 5120, 7168), (17, 7168, 9216),
               (21, 9216, 11264), (24, 11264, 12544)]

_CHUNK_OF = {}
for _ci, _blks in enumerate(_CHUNKS):
    for _k, _j in enumerate(_blks):
        _CHUNK_OF[_j] = (_ci, _BASE[_j] - _BASE[_blks[0]])


def _build_mlp_bass():
    import concourse.bass as bass
    import concourse.mybir as mybir
    import concourse.tile as tile

    F32 = mybir.dt.float32
    F16 = mybir.dt.float16
    AF = mybir.ActivationFunctionType

    nc = bass.Bass()
    xH_d = nc.dram_tensor("xH", [128, 25088], F16, kind="ExternalInput")
    Wp_d = nc.dram_tensor("Wp", [128, 192], F16, kind="ExternalInput")
    bp_d = nc.dram_tensor("bp", [64, 2], F32, kind="ExternalInput")
    out_d = nc.dram_tensor("out", [64, SHARD], F16, kind="ExternalOutput")

    with tile.TileContext(nc) as tc:
        with tc.tile_pool(name="const", bufs=1) as const, \
             tc.tile_pool(name="xs", bufs=5) as xs, \
             tc.tile_pool(name="hs", bufs=5) as hs, \
             tc.tile_pool(name="ps", bufs=8, space="PSUM") as ps_pool:

            Wt = const.tile([128, 192], F16, tag="W")
            bt = const.tile([64, 2], F32, tag="b")
            ho = []
            for i, (_, lo, hi) in enumerate(_OUT_SPLITS):
                ho_i = const.tile([64, hi - lo], F16, tag=f"ho{i}",
                                  name=f"ho{i}")
                ho.append(ho_i)

            xts = []

            def issue_chunk(ci):
                blks = _CHUNKS[ci]
                c0 = _BASE[blks[0]]
                cw = _BASE[blks[-1]] + 2 * _BLK[blks[-1]] - c0
                xt = xs.tile([128, 2048], F16, tag="xt", name=f"xt{ci}")
                nc.sync.dma_start(xt[:, :cw], xH_d[:, c0:c0 + cw])
                xts.append(xt)

            # x chunk 0 first (critical path), then the consts, then prefetch
            issue_chunk(0)
            nc.sync.dma_start(Wt[:], Wp_d[:])
            nc.sync.dma_start(bt[:], bp_d[:])
            issue_chunk(1)
            issue_chunk(2)

            def mm(out, lhsT, rhs, start, stop, reload):
                m = nc.tensor.matmul(out, lhsT=lhsT, rhs=rhs,
                                     start=start, stop=stop)
                if not reload:
                    m.ins.ldweights = False
                return m

            def bias_out(j, ps2s, use_scalar=False):
                # bias add for block j + out DMA when its staging chunk
                # completes. Output DMAs ride the GpSimd ring so they never
                # block input chunks on the sync ring (head-of-line).
                nw = _BLK[j]
                col = 512 * j
                for i, (after, lo, hi) in enumerate(_OUT_SPLITS):
                    if lo <= col < hi:
                        dst = ho[i][:, col - lo:col - lo + nw]
                        if use_scalar:
                            nc.scalar.activation(dst, ps2s[j][:, :nw],
                                                 AF.Identity, bias=bt[:, 1:2])
                        else:
                            nc.vector.tensor_scalar_add(dst, ps2s[j][:, :nw],
                                                        bt[:, 1:2])
                        if j == after:
                            nc.gpsimd.dma_start(out_d[:, lo:hi], ho[i][:])
                        break

            def emit_mm2(grp, h1s, ps2s, tail=False):
                # mm2 batch for a finished group, then bias adds + out DMAs.
                # In the tail (no relus left to delay) alternate DVE/scalar
                # to halve the serial bias chain.
                for k, j in enumerate(grp):
                    nw = _BLK[j]
                    ps2 = ps_pool.tile([64, 512], F32, tag="ps")
                    mm(ps2[:, :nw], Wt[0:64, 128:192], h1s[j][:, :nw],
                       True, True, k == 0)
                    ps2s[j] = ps2
                for k, j in enumerate(grp):
                    bias_out(j, ps2s, use_scalar=tail and k % 2 == 1)

            # mm1 groups of 4 blocks (2 chunks); mm2 of the previous group
            # is issued after the full mm1 pair phases (v3 ordering)
            groups = [list(range(g, min(g + 4, 25))) for g in range(0, 25, 4)]
            h1s = [None] * 25
            ps1s = [None] * 25
            ps2s = [None] * 25
            next_chunk = 3
            for gi, grp in enumerate(groups):
                while next_chunk <= min(_CHUNK_OF[grp[-1]][0] + 2,
                                        len(_CHUNKS) - 1):
                    issue_chunk(next_chunk)
                    next_chunk += 1
                for k, j in enumerate(grp):
                    nw = _BLK[j]
                    ci, off = _CHUNK_OF[j]
                    ps1 = ps_pool.tile([64, 512], F32, tag="ps")
                    ps1s[j] = ps1
                    mm(ps1[:, :nw], Wt[:, 0:64], xts[ci][:, off:off + nw],
                       True, False, k == 0)
                for k, j in enumerate(grp):
                    nw = _BLK[j]
                    ci, off = _CHUNK_OF[j]
                    mm(ps1s[j][:, :nw], Wt[:, 64:128],
                       xts[ci][:, off + nw:off + 2 * nw], False, True, k == 0)
                    h1 = hs.tile([64, 512], F16, tag="h1")
                    nc.scalar.activation(h1[:, :nw], ps1s[j][:, :nw], AF.Relu,
                                         bias=bt[:, 0:1])
                    h1s[j] = h1
                if gi >= 1:
                    emit_mm2(groups[gi - 1], h1s, ps2s,
                             tail=(gi == len(groups) - 1))
            emit_mm2(groups[-1], h1s, ps2s, tail=True)

    _dedupe_ldweights(nc)
    _legalize_waits(nc)
    return nc


def _mlp_kernel(x, W1, b1, W2, b2, scale):
    from concourse.bass_utils import run_bass_kernel_spmd
    nc = _build_mlp_bass()

    W1r = np.ascontiguousarray(
        W1.reshape(2, 128, 64).transpose(1, 0, 2).reshape(128, 128)
    ).astype(np.float16)
    W2s = (W2 * scale).astype(np.float16)
    Wp = np.zeros((128, 192), np.float16)
    Wp[:, 0:128] = W1r
    Wp[0:64, 128:192] = W2s
    bp = np.stack([b1, b2 * scale], axis=1).astype(np.float32)

    maps = []
    for c in range(P):
        xp = np.zeros((SHARD, 256), np.float16)
        xp[:NP] = x[c * NP:(c + 1) * NP]
        xH = np.empty((128, 25088), np.float16)
        xH[:, :24576] = (xp[:12288].reshape(24, 512, 2, 128)
                         .transpose(3, 0, 2, 1).reshape(128, 24576))
        xH[:, 24576:] = (xp[12288:].reshape(256, 2, 128)
                         .transpose(2, 1, 0).reshape(128, 512))
        maps.append({"xH": np.ascontiguousarray(xH), "Wp": Wp, "bp": bp})

    res = run_bass_kernel_spmd(nc, maps, core_ids=list(range(P)))

    full = np.empty((N, HID), np.float32)
    for c in range(P):
        full[c * NP:(c + 1) * NP] = res.results[c]["out"].T[:NP].astype(np.float32)
    return full


# ---------------------------------------------------------------------------
# general path: host-side graph preprocessing
# ---------------------------------------------------------------------------
def _vid_to_slotbase(v):
    t = v // 1024
    q = (v % 1024) // 128
    j = (v % 128) // 32
    m = v % 32
    return (32 * t + 8 * j + q) * 128 + 4 * m


def _build_structures(edge_index):
    rows = np.asarray(edge_index[0], dtype=np.int64)
    cols = np.asarray(edge_index[1], dtype=np.int64)
    outdeg = np.bincount(rows, minlength=N)

    cores = []
    for c in range(P):
        lo = c * NP
        sel = (cols >= lo) & (cols < lo + NP)
        e_src = rows[sel]
        e_dst = cols[sel] - lo
        order = np.argsort(e_dst, kind="stable")
        e_src = e_src[order]
        indeg = np.bincount(e_dst, minlength=NP)
        starts = np.zeros(NP + 1, dtype=np.int64)
        np.cumsum(indeg, out=starts[1:])
        vcnt = np.maximum(1, -(-indeg // L))
        perm = np.argsort(vcnt, kind="stable")
        cores.append(dict(e_src=e_src, starts=starts, indeg=indeg,
                          vcnt=vcnt, perm=perm))

    max_vc = max(int(c["vcnt"].max()) for c in cores)
    sizes = [SHARD]
    for p in range(1, max_vc):
        a = max(int((c["vcnt"] > p).sum()) for c in cores)
        sizes.append(min(SHARD, -(-(a + SHARD - NP) // 128) * 128))
    bases = np.concatenate([[0], np.cumsum(sizes)[:-1]]).astype(np.int64)
    acc_starts = np.array([0] + [SHARD - s for s in sizes[1:]], dtype=np.int64)
    NVID = int(sum(sizes))
    NVID_pad = -(-NVID // PSUM_VIDS) * PSUM_VIDS
    NSLOT = NVID_pad * L

    perm_pos = np.empty((P, NP), dtype=np.int64)
    for c in range(P):
        perm_pos[c][cores[c]["perm"]] = np.arange(NP)
    g_row = (np.repeat(np.arange(P), NP) * SHARD + perm_pos.ravel())

    all_idx, all_mask = [], []
    for c in range(P):
        cc = cores[c]
        idx = np.full(NSLOT, PAD_IDX, dtype=np.int32)
        for p in range(len(sizes)):
            sz, b, astart = sizes[p], int(bases[p]), int(acc_starts[p])
            r = np.arange(astart, astart + sz)
            v = b + (r - astart)
            real = r < NP
            d = cc["perm"][np.minimum(r, NP - 1)]
            has = real & (cc["vcnt"][d] > p)
            d_sel, v_sel = d[has], v[has]
            sbase = _vid_to_slotbase(v_sel)
            estart = cc["starts"][d_sel] + p * L
            cnt = np.minimum(cc["starts"][d_sel] + cc["indeg"][d_sel],
                             estart + L) - estart
            for i in range(L):
                sub = cnt > i
                src = cc["e_src"][estart[sub] + i]
                idx[sbase[sub] + i] = g_row[src]
        all_idx.append(idx)
        od = np.zeros(SHARD, dtype=np.int64)
        od[:NP] = outdeg[c * NP + cc["perm"]]
        all_mask.append((np.arange(64)[None, :] < od[:, None]).astype(np.float16))

    plan = dict(sizes=sizes, bases=bases, acc_starts=acc_starts,
                NVID=NVID, NVID_pad=NVID_pad, NSLOT=NSLOT)
    return cores, all_idx, all_mask, plan


def _plane_of_vid(plan, v0):
    bases, sizes = plan["bases"], plan["sizes"]
    p = int(np.searchsorted(bases, v0, side="right")) - 1
    if v0 >= bases[p] + sizes[p]:
        return None
    return p


def _dve_schedule(plan):
    ops = []
    n_tiles = plan["NVID_pad"] // PSUM_VIDS
    for t in range(n_tiles):
        run = None
        for q in range(8):
            v0 = 1024 * t + 128 * q
            p = _plane_of_vid(plan, v0) if v0 < plan["NVID"] else None
            if p is None:
                if run is not None:
                    ops.append(run)
                    run = None
                continue
            acc_row = int(plan["acc_starts"][p]) + (v0 - int(plan["bases"][p]))
            is_copy, chunk = (p == 0), acc_row // 128
            if (run is not None and run[3] == is_copy
                    and run[4] + (q - run[1]) == chunk):
                run = (t, run[1], q + 1, is_copy, run[4])
            else:
                if run is not None:
                    ops.append(run)
                run = (t, q, q + 1, is_copy, chunk)
        if run is not None:
            ops.append(run)
    return ops


# ---------------------------------------------------------------------------
# general-path Bass program (propagation up to k_eff steps)
# ---------------------------------------------------------------------------
def _build_bass(plan, sched, k_eff):
    import concourse.bass as bass
    import concourse.mybir as mybir
    import concourse.tile as tile
    from concourse.bass import IndirectOffsetOnAxis

    F32 = mybir.dt.float32
    F16 = mybir.dt.float16
    I32 = mybir.dt.int32
    AF = mybir.ActivationFunctionType
    OP = mybir.AluOpType

    NSLOT = plan["NSLOT"]
    groups_used = plan["NVID_pad"] // 32
    n_chunks = -(-groups_used // 128)
    n_ptiles = -(-groups_used // 32)
    sched_by_tile = {}
    for op in sched:
        sched_by_tile.setdefault(op[0], []).append(op)

    nc = bass.Bass()
    xT_d = nc.dram_tensor("xT", [256, SHARD], F32, kind="ExternalInput")
    W1_d = nc.dram_tensor("W1", [256, 64], F32, kind="ExternalInput")
    b1_d = nc.dram_tensor("b1", [64, 1], F32, kind="ExternalInput")
    W2_d = nc.dram_tensor("W2", [64, 64], F32, kind="ExternalInput")
    b2_d = nc.dram_tensor("b2", [64, 1], F32, kind="ExternalInput")
    chebMT_d = nc.dram_tensor("chebMT", [11, 11], F32, kind="ExternalInput")
    temp_d = nc.dram_tensor("temp", [11, 1], F32, kind="ExternalInput")
    ident_d = nc.dram_tensor("ident", [64, 64], F32, kind="ExternalInput")
    ones1_d = nc.dram_tensor("ones1", [128, 32], F16, kind="ExternalInput")
    ones2_d = nc.dram_tensor("ones2", [128, 32], F16, kind="ExternalInput")
    gidx_d = nc.dram_tensor("gidx", [128, NSLOT // 128], I32, kind="ExternalInput")
    mask_d = nc.dram_tensor("maskd", [SHARD, 64], F16, kind="ExternalInput")
    out_d = nc.dram_tensor("out", [SHARD, 64], F32, kind="ExternalOutput")

    with tile.TileContext(nc) as tc:
        with tc.tile_pool(name="big", bufs=1) as big, \
             tc.tile_pool(name="msgs", bufs=2) as msgs_pool, \
             tc.tile_pool(name="ps", bufs=4, space="PSUM") as ps_pool, \
             tc.tile_pool(name="sm", bufs=3) as sm, \
             tc.tile_pool(name="dram", bufs=1, space="DRAM") as dram:

            TxA = big.tile([128, NCH, 64], F32, tag="TxA")
            TxB = big.tile([128, NCH, 64], F32, tag="TxB")
            acc = big.tile([128, NCH, 64], F32, tag="acc")
            oacc = big.tile([128, NCH, 64], F32, tag="oacc")
            disw = big.tile([128, NCH, 64], F32, tag="disw")
            u16 = big.tile([128, NCH, 64], F16, tag="u16")
            idxt = big.tile([128, NSLOT // 128], I32, tag="idx")
            ones1 = big.tile([128, 32], F16, tag="ones1")
            ones2 = big.tile([128, 32], F16, tag="ones2")
            onesf = big.tile([128, 64], F32, tag="onesf")
            ones1x = big.tile([1, 128], F32, tag="ones1x")
            identt = big.tile([64, 64], F32, tag="ident")
            W1t = big.tile([128, 2, 64], F32, tag="W1")
            W2t = big.tile([64, 64], F32, tag="W2")
            b1t = big.tile([64, 1], F32, tag="b1")
            b2t = big.tile([64, 1], F32, tag="b2")
            coe_t = big.tile([128, 11], F32, tag="coe")
            dis_t = big.tile([128, NCH], F32, tag="dis")
            m1_t = big.tile([128, NCH], F32, tag="m1")

            nc.sync.dma_start(idxt[:], gidx_d[:])
            nc.sync.dma_start(W1t[:], W1_d[:].rearrange("(k p) h -> p k h", p=128))
            nc.sync.dma_start(W2t[:], W2_d[:])
            nc.sync.dma_start(b1t[:], b1_d[:])
            nc.sync.dma_start(b2t[:], b2_d[:])
            nc.sync.dma_start(identt[:], ident_d[:])
            nc.sync.dma_start(ones1[:], ones1_d[:])
            nc.sync.dma_start(ones2[:], ones2_d[:])
            nc.vector.memset(onesf[:], 1.0)
            nc.vector.memset(ones1x[:], 1.0)

            # coe = (2/(K+1)) * M @ temp, broadcast to all 128 partitions
            chebt = sm.tile([11, 11], F32, tag="chebt")
            tempt = sm.tile([11, 1], F32, tag="tempt")
            nc.sync.dma_start(chebt[:], chebMT_d[:])
            nc.sync.dma_start(tempt[:], temp_d[:])
            ps_coe = ps_pool.tile([1, 11], F32, tag="ps")
            nc.tensor.matmul(ps_coe[:], lhsT=tempt[:], rhs=chebt[:], start=True, stop=True)
            coe_row = sm.tile([1, 11], F32, tag="coerow")
            nc.vector.tensor_copy(coe_row[:], ps_coe[:])
            ps_coeb = ps_pool.tile([128, 11], F32, tag="ps")
            nc.tensor.matmul(ps_coeb[:], lhsT=ones1x[:], rhs=coe_row[:], start=True, stop=True)
            nc.vector.tensor_copy(coe_t[:], ps_coeb[:])

            # deg/dis from the out-degree unary mask
            maskt = msgs_pool.tile([128, NCH, 64], F16, tag="msgs")
            nc.sync.dma_start(maskt[:], mask_d[:].rearrange("(c p) f -> p c f", p=128))
            deg = sm.tile([128, NCH], F32, tag="deg")
            nc.vector.tensor_reduce(deg[:], maskt[:], axis=mybir.AxisListType.X, op=OP.add)
            nc.vector.tensor_scalar_min(m1_t[:], deg[:], 1.0)
            nc.vector.tensor_scalar_max(deg[:], deg[:], 0.5)
            rec = sm.tile([128, NCH], F32, tag="rec")
            nc.vector.reciprocal(rec[:], deg[:])
            nc.scalar.activation(dis_t[:], rec[:], AF.Sqrt)
            nc.vector.tensor_tensor(out=dis_t[:], in0=dis_t[:], in1=m1_t[:], op=OP.mult)
            for c in range(NCH):
                nc.scalar.activation(disw[:, c, :], onesf[:], AF.Copy,
                                     scale=dis_t[:, c:c + 1])

            # MLP: h = relu(x@W1+b1)@W2+b2, node-major into TxA
            nco = 0
            ci = 0
            for j in range(25):
                nw = 512 if j < 24 else 256
                ps1 = ps_pool.tile([64, 512], F32, tag="ps")
                for k in range(2):
                    xt = sm.tile([128, 512], F32, tag="xt")
                    nc.sync.dma_start(xt[:, :nw], xT_d[128 * k:128 * (k + 1), nco:nco + nw])
                    nc.tensor.matmul(ps1[:, :nw], lhsT=W1t[:, k, :], rhs=xt[:, :nw],
                                     start=(k == 0), stop=(k == 1))
                h1 = sm.tile([64, 512], F32, tag="h1")
                nc.scalar.activation(h1[:, :nw], ps1[:, :nw], AF.Relu, bias=b1t[:, 0:1])
                ps2 = ps_pool.tile([64, 512], F32, tag="ps")
                nc.tensor.matmul(ps2[:, :nw], lhsT=W2t[:], rhs=h1[:, :nw], start=True, stop=True)
                h2 = sm.tile([64, 512], F32, tag="h2")
                nc.vector.tensor_scalar_add(h2[:, :nw], ps2[:, :nw], b2t[:, 0:1])
                for cc in range(nw // 128):
                    pst = ps_pool.tile([128, 64], F32, tag="ps")
                    nc.tensor.transpose(pst[:], h2[:, 128 * cc:128 * (cc + 1)], identt[:])
                    nc.vector.tensor_copy(TxA[:, ci, :], pst[:])
                    ci += 1
                nco += nw

            # Chebyshev propagation steps
            u_bounce = dram.tile([SHARD, 64], F16, tag="ub")
            cur, prev = TxA, TxB
            for s in range(1, k_eff + 1):
                nc.vector.tensor_tensor(out=u16[:], in0=cur[:], in1=disw[:], op=OP.mult)
                nc.sync.dma_start(u_bounce[:].rearrange("(c p) f -> p c f", p=128), u16[:])
                ufull = dram.tile([P * SHARD, 64], F16, addr_space="Shared", tag=f"uf{s}")
                nc.gpsimd.collective_compute(
                    "AllGather", OP.bypass,
                    replica_groups=[list(range(P))],
                    ins=[u_bounce.opt()], outs=[ufull.opt()],
                )
                ones_t = ones1 if s == 1 else ones2
                for kk in range(n_chunks):
                    g0 = 128 * kk
                    gn = min(128, groups_used - g0)
                    mt = msgs_pool.tile([128, 128 * 64], F16, tag="msgs")
                    nc.gpsimd.indirect_dma_start(
                        out=mt[:, :gn * 64], out_offset=None,
                        in_=ufull[:],
                        in_offset=IndirectOffsetOnAxis(ap=idxt[:, g0:g0 + gn], axis=0),
                    )
                    for tt in range(4):
                        T = 4 * kk + tt
                        if T >= n_ptiles:
                            break
                        ps = ps_pool.tile([128, 512], F32, tag="ps")
                        for jj in range(4):
                            gbase = 32 * tt + 8 * jj
                            nq = min(8, groups_used - (32 * T + 8 * jj))
                            if nq <= 0:
                                break
                            nc.tensor.matmul(ps[32 * jj:32 * (jj + 1), :64 * nq],
                                             lhsT=ones_t[:],
                                             rhs=mt[:, gbase * 64:(gbase + nq) * 64],
                                             start=True, stop=True,
                                             tile_position=(0, 32 * jj))
                        for (_, qlo, qhi, is_copy, ch0) in sched_by_tile.get(T, []):
                            src = ps[:, 64 * qlo:64 * qhi]
                            dst = acc[:, ch0:ch0 + (qhi - qlo), :]
                            if is_copy:
                                nc.vector.tensor_copy(dst, src)
                            else:
                                nc.vector.tensor_tensor(out=dst, in0=dst, in1=src, op=OP.add)
                nc.vector.tensor_tensor(out=acc[:], in0=acc[:], in1=disw[:], op=OP.mult)
                if s == 1:
                    nc.vector.tensor_copy(prev[:], acc[:])
                    nc.vector.tensor_scalar(out=oacc[:], in0=cur[:],
                                            scalar1=coe_t[:, 0:1], scalar2=0.5,
                                            op0=OP.mult, op1=OP.mult)
                    nc.vector.tensor_scalar(out=acc[:], in0=prev[:],
                                            scalar1=coe_t[:, 1:2], scalar2=None,
                                            op0=OP.mult)
                    nc.vector.tensor_tensor(out=oacc[:], in0=oacc[:], in1=acc[:], op=OP.add)
                else:
                    nc.vector.tensor_tensor(out=prev[:], in0=acc[:], in1=prev[:], op=OP.subtract)
                    nc.vector.tensor_scalar(out=acc[:], in0=prev[:],
                                            scalar1=coe_t[:, s:s + 1], scalar2=None,
                                            op0=OP.mult)
                    nc.vector.tensor_tensor(out=oacc[:], in0=oacc[:], in1=acc[:], op=OP.add)
                cur, prev = prev, cur

            if k_eff == 0:
                nc.vector.tensor_scalar(out=oacc[:], in0=TxA[:],
                                        scalar1=coe_t[:, 0:1], scalar2=0.5,
                                        op0=OP.mult, op1=OP.mult)
            nc.sync.dma_start(out_d[:].rearrange("(c p) f -> p c f", p=128), oacc[:])

    _legalize_waits(nc)
    return nc


def _block_ones(v):
    o = np.zeros((128, 32), np.float16)
    for m in range(32):
        o[4 * m:4 * m + 4, m] = v
    return o


def _general_kernel(x, edge_index, W1, b1, W2, b2, temp, k_eff):
    from concourse.bass_utils import run_bass_kernel_spmd

    cores, all_idx, all_mask, plan = _build_structures(edge_index)
    sched = _dve_schedule(plan)
    nc = _build_bass(plan, sched, k_eff)

    chebMT = _cheb_MT()
    ident = np.eye(64, dtype=np.float32)
    o1, o2 = _block_ones(-1.0), _block_ones(-2.0)
    maps = []
    for c in range(P):
        cc = cores[c]
        xp = x[c * NP + cc["perm"]]
        xp = np.concatenate([xp, np.zeros((SHARD - NP, 256), np.float32)])
        maps.append({
            "xT": np.ascontiguousarray(xp.T),
            "W1": W1, "b1": b1.reshape(64, 1),
            "W2": W2, "b2": b2.reshape(64, 1),
            "chebMT": chebMT,
            "temp": temp.reshape(11, 1),
            "ident": ident,
            "ones1": o1, "ones2": o2,
            "gidx": np.ascontiguousarray(all_idx[c].reshape(-1, 128).T),
            "maskd": all_mask[c],
        })

    res = run_bass_kernel_spmd(nc, maps, core_ids=list(range(P)))

    full = np.zeros((N, HID), np.float32)
    for c in range(P):
        full[c * NP + cores[c]["perm"]] = res.results[c]["out"][:NP]
    return full


# ---------------------------------------------------------------------------
# public entry point
# ---------------------------------------------------------------------------
def kernel(x, edge_index, W1, b1, W2, b2, temp):
    _install_patches()

    x = np.asarray(x, np.float32)
    W1 = np.asarray(W1, np.float32)
    b1 = np.asarray(b1, np.float32)
    W2 = np.asarray(W2, np.float32)
    b2 = np.asarray(b2, np.float32)
    temp = np.asarray(temp, np.float32)

    # significant Chebyshev orders, computed on host in fp64
    coe = (2.0 / (K + 1)) * (_cheb_M64() @ temp.astype(np.float64))
    thr = 1e-6 * max(np.abs(coe).max(), 1e-30)
    sig = np.nonzero(np.abs(coe) > thr)[0]
    k_eff = int(sig.max()) if (sig.size and sig.max() >= 1) else 0

    if k_eff == 0:
        return _mlp_kernel(x, W1, b1, W2, b2, float(coe[0] / 2.0))
    return _general_kernel(x, edge_index, W1, b1, W2, b2, temp, k_eff)
